# revision 1
# baseline (speedup 1.0000x reference)
"""CantorAttention Trainium2 kernel (8 NeuronCores).

Architecture
------------
The reference gathers K=64 routed keys/values per query (with +-1 smoothing)
and does sparse attention. Gathering k/v rows on TRN2 is bandwidth-doomed
(gathered tensor is 256MB); instead we *rematerialize densely* on the PE:

  smoothing commutes with the gather:  k_g[s,i] = k_s[r[s,i]] where
  k_s[j] = 0.5*k[j] + 0.25*(k[max(j-1,0)] + k[min(j+1,S-1)])

  softmax over 64 slots (with duplicate routes) == dense masked softmax with
  multiplicity weights M[s,j] = #{i : r[s,i] = j}:

    out[s] = sum_j M[s,j] * exp(zd[s,j]) * v_s[j] / sum_j M[s,j] * exp(zd[s,j])
    zd[s,j] = scale * q[s] . k_s[j]

so everything becomes dense matmuls + one dense exp + one dense mask-multiply.

Sharding: phase 1 = one head per core (scores/softmax/AV, outputs
unnormalized head outputs + softmax denominators); phase 2 = output
projection, sharded over sequence (each core takes 256 query positions,
all heads), avoiding any on-device collective.

Layout notes: phase-1 keeps everything transposed ([dim, seq] /
[key-block, seq]) so the sequence axis is always the matmul moving dim and
the smoothing shift is a free-dim offset. exp() runs with no max-subtract:
zd = q.k_s/8 with ~N(0,1) entries, |zd| << 80, so fp32/bf16 exp is safe.
"""
import sys

sys.path.insert(0, "/opt/trn_rl_repo")

import numpy as np
import ml_dtypes

import concourse.bass as bass
import concourse.bacc as bacc
import concourse.mybir as mybir
from concourse import tile
from concourse import bass_utils

BF16 = mybir.dt.bfloat16
F32 = mybir.dt.float32
Exp = mybir.ActivationFunctionType.Exp
Copy = mybir.ActivationFunctionType.Copy
ADD = mybir.AluOpType.add
MULT = mybir.AluOpType.mult

S = 2048  # sequence length
D = 512  # model dim
H = 8  # heads
HD = 64  # head dim
KN = 64  # routed neighbors per query
NCORES = 8
SC = S // 512  # moving-dim chunks of 512
JB = S // 128  # key blocks of 128

_nc1 = None
_nc2 = None


def _build_phase1():
    nc = bacc.Bacc("TRN2", target_bir_lowering=False, debug=False, num_devices=NCORES)
    xt_d = nc.dram_tensor("xt", [128, 4 * S], BF16, kind="ExternalInput").ap()
    wq_d = nc.dram_tensor("wq", [128, 4 * HD], BF16, kind="ExternalInput").ap()
    wkv_d = nc.dram_tensor("wkv", [128, 4 * 2 * HD], BF16, kind="ExternalInput").ap()
    bq_d = nc.dram_tensor("bq", [HD, 1], F32, kind="ExternalInput").ap()
    bkv_d = nc.dram_tensor("bkv", [2 * HD, 1], F32, kind="ExternalInput").ap()
    mt_d = nc.dram_tensor("mt", [S, S], BF16, kind="ExternalInput").ap()
    id64_d = nc.dram_tensor("id64", [128, HD], BF16, kind="ExternalInput").ap()
    outu_d = nc.dram_tensor("outu", [HD + 1, S], F32, kind="ExternalOutput").ap()

    with tile.TileContext(nc) as tc:
        with (
            tc.tile_pool(name="const", bufs=1) as const,
            tc.tile_pool(name="work", bufs=1) as work,
            tc.tile_pool(name="mstream", bufs=6) as mstream,
            tc.tile_pool(name="estream", bufs=4) as estream,
            tc.tile_pool(name="ps_big", bufs=1, space="PSUM") as ps_big,
        ):
            xt = const.tile([128, 4 * S], BF16)
            wq = const.tile([128, 4 * HD], BF16)
            wkv = const.tile([128, 4 * 2 * HD], BF16)
            bq = const.tile([HD, 1], F32)
            bkv = const.tile([2 * HD, 1], F32)
            id64 = const.tile([128, HD], BF16)
            nc.sync.dma_start(wkv[:], wkv_d[:])
            nc.sync.dma_start(wq[:], wq_d[:])
            nc.sync.dma_start(bq[:], bq_d[:])
            nc.sync.dma_start(bkv[:], bkv_d[:])
            nc.sync.dma_start(id64[:], id64_d[:])
            for c in range(4):
                nc.sync.dma_start(
                    xt[:, 2048 * c : 2048 * (c + 1)], xt_d[:, 2048 * c : 2048 * (c + 1)]
                )

            qt = work.tile([128, S], BF16)  # q^T * (1/16); rows 64-127 = copy
            kx = work.tile([128, S], BF16)  # rows 64-127 = copy of k~^T
            kvpad = work.tile([128, S + 2], F32)  # rows 0-63 k^T, 64-127 v^T
            kvs = work.tile([128, S], BF16)  # smoothed k~^T / v~^T
            vaug = work.tile([128, JB * 128], BF16)  # v~ blocks + ones col (128-stride aligned)

            # PSUM: 8 banks. Two half-width zd tiles (2 banks each) double-
            # buffer the scores->exp pipeline; outu (4 banks) accumulates AV.
            # All are also reused as projection accumulators via slices.
            zd_a = ps_big.tile([128, S // 2], F32)
            zd_b = ps_big.tile([128, S // 2], F32)
            outu_ps = ps_big.tile([HD + 1, S], F32)

            # --- kv projection (transposed): psum[c,s] = sum_d W[d,c] xT[d,s]
            # biases are folded into the PSUM->SBUF copies (per-partition adds)
            for sc in range(SC):
                kv_ps = (zd_a if sc % 2 == 0 else zd_b)[:, 512 * (sc // 2) : 512 * (sc // 2 + 1)]
                for c in range(4):
                    rhs = xt[:, 2048 * c + 512 * sc : 2048 * c + 512 * (sc + 1)]
                    nc.tensor.matmul(
                        kv_ps, wkv[:, 128 * c : 128 * (c + 1)], rhs, start=(c == 0), stop=(c == 3)
                    )
                nc.vector.tensor_scalar_add(
                    kvpad[:, 1 + 512 * sc : 1 + 512 * (sc + 1)], kv_ps, bkv[:]
                )

            # smoothing leads the DVE queue (ahead of the q bias-copies) so
            # the transposes it gates start ASAP; the q projection matmuls
            # fill the PE meanwhile, keeping the HAM clock-gate warm.
            nc.vector.tensor_copy(kvpad[:, 0:1], kvpad[:, 1:2])
            nc.vector.tensor_copy(kvpad[:, S + 1 : S + 2], kvpad[:, S : S + 1])
            tsm = work.tile([128, S], F32)
            # t = 0.5*shiftL + base ; kvs = 0.5*shiftR + t   (kvs = 2 * smoothed)
            nc.vector.scalar_tensor_tensor(
                tsm[:], kvpad[:, 0:S], 0.5, kvpad[:, 1 : S + 1], MULT, ADD
            )
            nc.vector.scalar_tensor_tensor(
                kvs[:], kvpad[:, 2 : S + 2], 0.5, tsm[:], MULT, ADD
            )

            for sc in range(SC):
                q_ps = outu_ps[0:HD, 512 * sc : 512 * (sc + 1)]
                for c in range(4):
                    rhs = xt[:, 2048 * c + 512 * sc : 2048 * c + 512 * (sc + 1)]
                    nc.tensor.matmul(
                        q_ps, wq[:, HD * c : HD * (c + 1)], rhs, start=(c == 0), stop=(c == 3)
                    )
                nc.vector.tensor_scalar_add(qt[0:HD, 512 * sc : 512 * (sc + 1)], q_ps, bq[:])

            # duplicate k~^T and q^T into partitions 64-127 so score matmuls
            # for odd key-blocks can run in PE row-group 64-127 concurrently
            # with even key-blocks in rows 0-63 (row-packed pairs)
            nc.scalar.dma_start(kx[HD:128, :], kvs[0:HD, :])
            nc.scalar.dma_start(qt[HD:128, :], qt[0:HD, :])

            # --- v~ blocks transposed into [j-in-block, hd] layout + ones col.
            # PE-mode transpose (not DMA): keeps the PE busy and off the DMA
            # queues; outputs staged through the (currently free) zd PSUM.
            for jb in range(JB):
                tp = (zd_a if jb % 2 == 0 else zd_b)[
                    :, 64 * (jb // 2) : 64 * (jb // 2) + HD
                ].bitcast(BF16)[:, 0:HD]
                nc.tensor.transpose(tp, kvs[HD:128, 128 * jb : 128 * (jb + 1)], id64[HD:128, :])
                nc.vector.tensor_copy(vaug[:, 128 * jb : 128 * jb + HD], tp)
                nc.gpsimd.memset(vaug[:, 128 * jb + HD : 128 * jb + HD + 1], 1.0)

            # --- dense masked attention: key-block PAIRS x seq-halves.
            # jb0 scores run in PE rows 0-63 while jb1 runs rows 64-127
            # (row-group packing); exp(zd_a) frees zd_a while exp(zd_b) and
            # the AV matmuls still overlap the next pair's scores.
            H2 = S // 2
            for u in range(JB):
                jp, sh = u // 2, u % 2
                jb0, jb1 = 2 * jp, 2 * jp + 1
                for c in range(2):
                    nc.tensor.matmul(
                        zd_a[:, 512 * c : 512 * (c + 1)],
                        kvs[0:HD, 128 * jb0 : 128 * (jb0 + 1)],
                        qt[0:HD, H2 * sh + 512 * c : H2 * sh + 512 * (c + 1)],
                        start=True,
                        stop=True,
                    )
                    nc.tensor.matmul(
                        zd_b[:, 512 * c : 512 * (c + 1)],
                        kx[HD:128, 128 * jb1 : 128 * (jb1 + 1)],
                        qt[HD:128, H2 * sh + 512 * c : H2 * sh + 512 * (c + 1)],
                        start=True,
                        stop=True,
                    )
                for half, (zd, jb) in enumerate(((zd_a, jb0), (zd_b, jb1))):
                    e_bf = estream.tile([128, H2], BF16, tag="e")
                    nc.scalar.activation(e_bf[:], zd[:], Exp)
                    m_bf = mstream.tile([128, H2], BF16, tag="m")
                    nc.sync.dma_start(
                        m_bf[:], mt_d[128 * jb : 128 * (jb + 1), H2 * sh : H2 * (sh + 1)]
                    )
                    nc.vector.tensor_mul(e_bf[:], e_bf[:], m_bf[:])
                    for c in range(2):
                        nc.tensor.matmul(
                            outu_ps[:, H2 * sh + 512 * c : H2 * sh + 512 * (c + 1)],
                            vaug[:, 128 * jb : 128 * jb + HD + 1],
                            e_bf[:, 512 * c : 512 * (c + 1)],
                            start=(jb == 0),
                            stop=(jb == JB - 1),
                        )
            outu_sb = work.tile([HD + 1, S], F32)
            for c in range(SC):
                nc.vector.tensor_copy(
                    outu_sb[:, 512 * c : 512 * (c + 1)], outu_ps[:, 512 * c : 512 * (c + 1)]
                )
                eng = nc.sync if c % 2 == 0 else nc.scalar
                eng.dma_start(
                    outu_d[:, 512 * c : 512 * (c + 1)], outu_sb[:, 512 * c : 512 * (c + 1)]
                )
    nc.compile()
    return nc


def _build_phase2():
    nc = bacc.Bacc("TRN2", target_bir_lowering=False, debug=False, num_devices=NCORES)
    SS = S // NCORES  # 256 query positions per core
    u_d = nc.dram_tensor("u", [128, 4 * SS], F32, kind="ExternalInput").ap()
    l_d = nc.dram_tensor("l", [H, SS], F32, kind="ExternalInput").ap()
    wo_d = nc.dram_tensor("wo", [128, 4 * D], BF16, kind="ExternalInput").ap()
    bo_d = nc.dram_tensor("bo", [1, D], BF16, kind="ExternalInput").ap()
    bl_d = nc.dram_tensor("bl", [H, D], F32, kind="ExternalInput").ap()
    y_d = nc.dram_tensor("y", [SS, D], F32, kind="ExternalOutput").ap()

    with tile.TileContext(nc) as tc:
        with (
            tc.tile_pool(name="sb", bufs=1) as sb,
            tc.tile_pool(name="ps", bufs=2, space="PSUM") as ps,
        ):
            u = sb.tile([128, 4 * SS], F32)
            lt = sb.tile([H, SS], F32)
            wo = sb.tile([128, 4 * D], BF16)
            bo = sb.tile([1, D], BF16)
            bl = sb.tile([H, D], F32)
            onescol = sb.tile([1, 128], BF16)
            nc.sync.dma_start(lt[:], l_d[:])
            nc.sync.dma_start(bl[:], bl_d[:])
            nc.sync.dma_start(bo[:], bo_d[:])
            for c in range(4):
                nc.sync.dma_start(u[:, SS * c : SS * (c + 1)], u_d[:, SS * c : SS * (c + 1)])
                nc.scalar.dma_start(wo[:, D * c : D * (c + 1)], wo_d[:, D * c : D * (c + 1)])
            nc.gpsimd.memset(onescol[:], 1.0)

            rl = sb.tile([H, SS], F32)
            nc.vector.reciprocal(rl[:], lt[:])

            # broadcast 1/l to all 64 rows of each head block: rl_ps[r, s]
            rl_ps = ps.tile([128, 4 * SS], F32)
            for c in range(4):
                nc.tensor.matmul(
                    rl_ps[:, SS * c : SS * (c + 1)],
                    bl[:, 128 * c : 128 * (c + 1)],
                    rl[:],
                    start=True,
                    stop=True,
                )
            un = sb.tile([128, 4 * SS], BF16)
            for c in range(4):
                nc.vector.tensor_mul(
                    un[:, SS * c : SS * (c + 1)],
                    u[:, SS * c : SS * (c + 1)],
                    rl_ps[:, SS * c : SS * (c + 1)],
                )

            for sb2 in range(SS // 128):
                y_ps = ps.tile([128, D], F32, tag="yps")
                for c in range(4):
                    nc.tensor.matmul(
                        y_ps[:],
                        un[:, SS * c + 128 * sb2 : SS * c + 128 * (sb2 + 1)],
                        wo[:, D * c : D * (c + 1)],
                        start=(c == 0),
                        stop=False,
                    )
                nc.tensor.matmul(y_ps[:], onescol[:], bo[:], start=False, stop=True)
                y_sb = sb.tile([128, D], F32, tag="ysb")
                nc.vector.tensor_copy(y_sb[:], y_ps[:])
                nc.sync.dma_start(y_d[128 * sb2 : 128 * (sb2 + 1), :], y_sb[:])
    nc.compile()
    return nc


def _prep_phase1_inputs(x, routes, W_qkv, b_qkv):
    x2 = np.asarray(x, dtype=np.float32).reshape(S, D)
    xt = np.ascontiguousarray(x2.T)  # [D, S]
    xt_r = (
        xt.reshape(4, 128, S).transpose(1, 0, 2).reshape(128, 4 * S).astype(ml_dtypes.bfloat16)
    )
    W = np.asarray(W_qkv, dtype=np.float32)
    b = np.asarray(b_qkv, dtype=np.float32)
    r = np.asarray(routes)
    M = np.zeros((S, S), dtype=np.float32)
    np.add.at(M, (np.arange(S)[:, None], r), 1.0)
    mt = np.ascontiguousarray(M.T).astype(ml_dtypes.bfloat16)

    idf = np.zeros((128, HD), dtype=ml_dtypes.bfloat16)
    idf[HD:128, :] = np.eye(HD, dtype=ml_dtypes.bfloat16)
    in_maps = []
    for h in range(NCORES):
        wq = W[:, h * HD : (h + 1) * HD] * (1.0 / 16.0)
        wk = W[:, D + h * HD : D + (h + 1) * HD]
        wv = W[:, 2 * D + h * HD : 2 * D + (h + 1) * HD]
        wkv = np.concatenate([wk, wv], axis=1)  # [D, 128]
        bq = b[h * HD : (h + 1) * HD] * (1.0 / 16.0)
        bkv = np.concatenate(
            [b[D + h * HD : D + (h + 1) * HD], b[2 * D + h * HD : 2 * D + (h + 1) * HD]]
        )
        in_maps.append(
            {
                "xt": xt_r,
                "wq": wq.reshape(4, 128, HD).transpose(1, 0, 2).reshape(128, 4 * HD).astype(ml_dtypes.bfloat16),
                "wkv": wkv.reshape(4, 128, 2 * HD).transpose(1, 0, 2).reshape(128, 8 * HD).astype(ml_dtypes.bfloat16),
                "bq": np.ascontiguousarray(bq.reshape(HD, 1), dtype=np.float32),
                "bkv": np.ascontiguousarray(bkv.reshape(2 * HD, 1), dtype=np.float32),
                "mt": mt,
                "id64": idf,
            }
        )
    return in_maps


def _prep_phase2_inputs(outs, W_out, b_out):
    SS = S // NCORES
    U = np.concatenate([o[0:HD, :] for o in outs], axis=0)  # [512, S] f32
    L = np.stack([o[HD, :] for o in outs], axis=0)  # [8, S]
    wo = (0.5 * np.asarray(W_out, dtype=np.float32)).astype(ml_dtypes.bfloat16)
    wo_r = np.ascontiguousarray(wo).reshape(4, 128, D).transpose(1, 0, 2).reshape(128, 4 * D)
    bo = np.asarray(b_out, dtype=np.float32).reshape(1, D).astype(ml_dtypes.bfloat16)
    bl = np.zeros((H, D), dtype=np.float32)
    for h in range(H):
        bl[h, h * HD : (h + 1) * HD] = 1.0
    in_maps = []
    for c in range(NCORES):
        Uc = U[:, c * SS : (c + 1) * SS]
        u_r = np.ascontiguousarray(Uc).reshape(4, 128, SS).transpose(1, 0, 2).reshape(128, 4 * SS)
        in_maps.append(
            {
                "u": np.ascontiguousarray(u_r),
                "l": np.ascontiguousarray(L[:, c * SS : (c + 1) * SS]),
                "wo": np.ascontiguousarray(wo_r),
                "bo": bo,
                "bl": bl,
            }
        )
    return in_maps


def _run(nc, in_maps, **kw):
    return bass_utils.run_bass_kernel_spmd(nc, in_maps, list(range(NCORES)), **kw)


def kernel(x, routes, W_qkv, b_qkv, W_out, b_out, _timing=None):
    global _nc1, _nc2
    if _nc1 is None:
        _nc1 = _build_phase1()
    if _nc2 is None:
        _nc2 = _build_phase2()

    in1 = _prep_phase1_inputs(x, routes, W_qkv, b_qkv)
    r1 = _run(_nc1, in1)
    outs = [r1.results[h]["outu"] for h in range(NCORES)]

    in2 = _prep_phase2_inputs(outs, W_out, b_out)
    r2 = _run(_nc2, in2)
    SS = S // NCORES
    y = np.concatenate([r2.results[c]["y"] for c in range(NCORES)], axis=0)

    if _timing is not None:
        _timing["r1"] = r1
        _timing["r2"] = r2
        _timing["in1"] = in1
        _timing["in2"] = in2
    return y.reshape(1, S, D).astype(np.float32)



# revision 9
# speedup vs baseline: 2.5103x; 2.5103x over previous
"""CantorAttention Trainium2 kernel (8 NeuronCores) — banded single-phase.

Key ideas
---------
1. The Cantor function is monotone, so sorting BOTH queries and keys by
   Cantor coordinate makes each query's 64 routed keys (ties included)
   fall in a <=127-wide contiguous window of sorted key order. The
   routed gather + duplicate-route softmax then becomes a *banded dense
   masked attention*: per core, 256 sorted queries attend to a 384-wide
   key window (3 blocks of 128) with a multiplicity mask M.
2. The +-1 neighbor smoothing is linear and commutes with the k/v
   projection, so it is applied to x on the HOST (exact, f32):
   k~ = W_k^T (A x) — the device never smooths.
3. Sequence sharding (256 queries x all 8 heads per core) makes the
   output projection local: one NEFF, no collective, no second phase.

Per-core device program: project q (all heads, its queries) and k/v
(all heads, its key window), dense scores over 3 key blocks, exp (no
max-subtract; |zd|<~6), multiply by multiplicity mask, AV with a ones
column for the softmax denominator, normalize via a small f32r
broadcast matmul, local out-projection, write its [256, 512] slice.
Host unpermutes rows at the end.
"""
import sys

sys.path.insert(0, "/opt/trn_rl_repo")

import numpy as np
import ml_dtypes

import concourse.bass as bass
import concourse.bacc as bacc
import concourse.mybir as mybir
from concourse import tile
from concourse import bass_utils

BF16 = mybir.dt.bfloat16
F32 = mybir.dt.float32
F32R = mybir.dt.float32r
Exp = mybir.ActivationFunctionType.Exp

S = 2048
D = 512
H = 8
HD = 64
NCORES = 8
SS = S // NCORES  # 256 queries per core
W = 384  # key-window width per core (3 blocks of 128)
NJB = W // 128

_nc = None


def _cantor_coords(seq_len, depth=8):
    x = np.arange(seq_len, dtype=np.float64) / max(1, seq_len - 1)
    x = np.clip(x, 1e-06, 1.0 - 1e-06)
    c = np.zeros_like(x)
    factor = 0.5
    for _ in range(depth):
        xs = x * 3.0
        digit = xs.astype(np.int64)
        x = xs - digit
        c = c + (digit == 2).astype(np.float64) * factor
        factor *= 0.5
    return np.clip(c, 0.0, 1.0)


def _build(has_bv, has_bo):
    nc = bacc.Bacc("TRN2", target_bir_lowering=False, debug=False, num_devices=NCORES)
    xq_d = nc.dram_tensor("xq", [128, 4 * SS], BF16, kind="ExternalInput").ap()
    xs_d = nc.dram_tensor("xs", [128, 4 * W], BF16, kind="ExternalInput").ap()
    wq_d = nc.dram_tensor("wq", [128, 2048], BF16, kind="ExternalInput").ap()
    wk_d = nc.dram_tensor("wk", [128, 2048], BF16, kind="ExternalInput").ap()
    wv_d = nc.dram_tensor("wv", [128, 2048], BF16, kind="ExternalInput").ap()
    wo_d = nc.dram_tensor("wo", [128, 2048], BF16, kind="ExternalInput").ap()
    m_d = nc.dram_tensor("m", [128, NJB * SS], BF16, kind="ExternalInput").ap()
    sel_d = nc.dram_tensor("sel", [8, 512], BF16, kind="ExternalInput").ap()
    bq_d = nc.dram_tensor("bq", [128, 4], F32, kind="ExternalInput").ap()
    bk_d = nc.dram_tensor("bk", [128, 4], F32, kind="ExternalInput").ap()
    bv_d = nc.dram_tensor("bv", [1, 512], BF16, kind="ExternalInput").ap()
    bo_d = nc.dram_tensor("bo", [1, 512], BF16, kind="ExternalInput").ap()
    y_d = nc.dram_tensor("y", [SS, D], F32, kind="ExternalOutput").ap()

    with tile.TileContext(nc) as tc:
        with (
            tc.tile_pool(name="const", bufs=1) as const,
            tc.tile_pool(name="work", bufs=1) as work,
            tc.tile_pool(name="estream", bufs=3) as estream,
            tc.tile_pool(name="ps_proj", bufs=2, space="PSUM") as ps_proj,
            tc.tile_pool(name="ps_zd", bufs=2, space="PSUM") as ps_zd,
            tc.tile_pool(name="ps_o", bufs=2, space="PSUM") as ps_o,
        ):
            xq = const.tile([128, 4 * SS], BF16)
            xs = const.tile([128, 4 * W], BF16)
            wq = const.tile([128, 2048], BF16)
            wk = const.tile([128, 2048], BF16)
            wv = const.tile([128, 2048], BF16)
            wo = const.tile([128, 2048], BF16)
            m = const.tile([128, NJB * SS], BF16)
            sel = const.tile([8, 512], BF16)
            bq = const.tile([128, 4], F32)
            bk = const.tile([128, 4], F32)
            bv = const.tile([1, 512], BF16)
            bo = const.tile([1, 512], BF16)
            ones1 = const.tile([1, 128], BF16)
            nc.sync.dma_start(xq[:], xq_d[:])
            nc.sync.dma_start(xs[:], xs_d[:])
            nc.sync.dma_start(wq[:], wq_d[:])
            nc.sync.dma_start(wk[:], wk_d[:])
            nc.scalar.dma_start(wv[:], wv_d[:])
            nc.scalar.dma_start(wo[:], wo_d[:])
            nc.scalar.dma_start(m[:], m_d[:])
            nc.gpsimd.dma_start(sel[:], sel_d[:])
            nc.gpsimd.dma_start(bq[:], bq_d[:])
            nc.gpsimd.dma_start(bk[:], bk_d[:])
            if has_bv:
                nc.gpsimd.dma_start(bv[:], bv_d[:])
            if has_bo:
                nc.gpsimd.dma_start(bo[:], bo_d[:])
            nc.gpsimd.memset(ones1[:], 1.0)

            qt = work.tile([128, 4 * SS], BF16)  # tile hp: [q_{2hp}|q_{2hp+1}] x queries
            kt = work.tile([128, 4 * W], BF16)  # tile hp: [k_{2hp}|k_{2hp+1}] x window
            # vjd[jb]: [128 j, 8*65]; head h cols h*65..h*65+65 = [v|1]
            vjd = [work.tile([128, 8 * 65], BF16, name=f"vjd{j}") for j in range(NJB)]
            uf = work.tile([128, 4 * SS], F32)  # u staging, hp layout
            lf = work.tile([65, 4 * SS], F32)  # row 64: even-head l per hp block
            la = work.tile([8, SS], F32)
            rl = work.tile([8, SS], F32)
            rlb = work.tile([8, SS], BF16)
            un = work.tile([128, 4 * SS], BF16)

            # --- q projection (4 head-pair tiles)
            for hp in range(4):
                pp = ps_proj.tile([128, 512], F32, tag="pp")
                pq = pp[:, 0:SS]
                for c in range(4):
                    nc.tensor.matmul(
                        pq[:],
                        wq[:, (hp * 4 + c) * 128 : (hp * 4 + c + 1) * 128],
                        xq[:, c * SS : (c + 1) * SS],
                        start=(c == 0),
                        stop=(c == 3),
                    )
                nc.vector.tensor_scalar_add(qt[:, hp * SS : (hp + 1) * SS], pq[:], bq[:, hp : hp + 1])

            # --- k projection (4 head-pair tiles over the window)
            for hp in range(4):
                pp = ps_proj.tile([128, 512], F32, tag="pp")
                pk = pp[:, 0:W]
                for c in range(4):
                    nc.tensor.matmul(
                        pk[:],
                        wk[:, (hp * 4 + c) * 128 : (hp * 4 + c + 1) * 128],
                        xs[:, c * W : (c + 1) * W],
                        start=(c == 0),
                        stop=(c == 3),
                    )
                nc.vector.tensor_scalar_add(kt[:, hp * W : (hp + 1) * W], pk[:], bk[:, hp : hp + 1])

            # --- v projection, direct [j, (h,d)] orientation per block
            for jb in range(NJB):
                pv = ps_proj.tile([128, 512], F32, tag="pp")
                for c in range(4):
                    nc.tensor.matmul(
                        pv[:],
                        xs[:, c * W + jb * 128 : c * W + (jb + 1) * 128],
                        wv[:, c * 512 : (c + 1) * 512],
                        start=(c == 0),
                        stop=(c == 3 and not has_bv),
                    )
                if has_bv:
                    nc.tensor.matmul(pv[:], ones1[:], bv[:], start=False, stop=True)
                for h in range(H):
                    off = h * 65
                    nc.vector.tensor_copy(vjd[jb][:, off : off + 64], pv[:, h * 64 : (h + 1) * 64])
                    nc.gpsimd.memset(vjd[jb][:, off + 64 : off + 65], 1.0)

            # --- banded attention, software-pipelined by one head
            zds = {}
            es = {}
            pos = {}

            def scores(h):
                hp, r = h // 2, 64 * (h % 2)
                zd = ps_zd.tile([128, NJB * SS], F32, tag="zd")
                for jb in range(NJB):
                    nc.tensor.matmul(
                        zd[:, jb * SS : (jb + 1) * SS],
                        kt[r : r + 64, hp * W + jb * 128 : hp * W + (jb + 1) * 128],
                        qt[r : r + 64, hp * SS : (hp + 1) * SS],
                        start=True,
                        stop=True,
                    )
                e = estream.tile([128, NJB * SS], BF16, tag="e")
                nc.scalar.activation(e[:], zd[:], Exp)
                nc.vector.tensor_mul(e[:], e[:], m[:])
                zds[h], es[h] = zd, e

            def av(h):
                e = es[h]
                po = ps_o.tile([128, SS], F32, tag="po")
                for jb in range(NJB):
                    nc.tensor.matmul(
                        po[0:65, :],
                        vjd[jb][:, h * 65 : h * 65 + 65],
                        e[:, jb * SS : (jb + 1) * SS],
                        start=(jb == 0),
                        stop=(jb == NJB - 1),
                    )
                hp = h // 2
                if h % 2 == 0:
                    nc.vector.tensor_copy(uf[0:64, hp * SS : (hp + 1) * SS], po[0:64, :])
                    nc.vector.tensor_copy(lf[64:65, hp * SS : (hp + 1) * SS], po[64:65, :])
                else:
                    # partition shift 0..64 -> 64..128 is DMA-only
                    ustg = work.tile([65, SS], F32, tag="ustg", bufs=2)
                    nc.vector.tensor_copy(ustg[:], po[0:65, :])
                    nc.scalar.dma_start(uf[64:128, hp * SS : (hp + 1) * SS], ustg[0:64, :])
                    nc.scalar.dma_start(la[4 + hp : 5 + hp, :], ustg[64:65, :])

            for h in range(H):
                scores(h)
                if h > 0:
                    av(h - 1)
            av(H - 1)

            # --- normalization: gather l, reciprocal, f32r broadcast matmul
            nc.sync.dma_start(la[0:4, :], lf[64:65, :])
            nc.vector.reciprocal(rl[:], la[:])
            nc.vector.tensor_copy(rlb[:], rl[:])
            for hp in range(4):
                pp = ps_proj.tile([128, 512], F32, tag="pp")
                prl = pp[:, 0:SS]
                nc.tensor.matmul(
                    prl[:],
                    sel[:, hp * 128 : (hp + 1) * 128],
                    rlb[:],
                    start=True,
                    stop=True,
                )
                nc.vector.tensor_mul(
                    un[:, hp * SS : (hp + 1) * SS], uf[:, hp * SS : (hp + 1) * SS], prl[:]
                )

            # --- output projection (2 query tiles of 128)
            for q2 in range(2):
                py = ps_proj.tile([128, 512], F32, tag="pp")
                for hp in range(4):
                    nc.tensor.matmul(
                        py[:],
                        un[:, hp * SS + q2 * 128 : hp * SS + (q2 + 1) * 128],
                        wo[:, hp * 512 : (hp + 1) * 512],
                        start=(hp == 0),
                        stop=(hp == 3 and not has_bo),
                    )
                if has_bo:
                    nc.tensor.matmul(py[:], ones1[:], bo[:], start=False, stop=True)
                ysb = work.tile([128, 512], F32, tag="ysb", bufs=2)
                nc.vector.tensor_copy(ysb[:], py[:])
                nc.sync.dma_start(y_d[q2 * 128 : (q2 + 1) * 128, :], ysb[:])
    nc.compile()
    return nc


def _to_chunked(a128xN, nchunks):
    """[128*nchunks, N] -> [128, nchunks*N] with chunk c at cols c*N."""
    n = a128xN.shape[1]
    return (
        a128xN.reshape(nchunks, 128, n).transpose(1, 0, 2).reshape(128, nchunks * n)
    )


def _prep(x, routes, W_qkv, b_qkv, W_out, b_out):
    x2 = np.asarray(x, dtype=np.float32).reshape(S, D)
    r = np.asarray(routes).astype(np.int64)
    Wf = np.asarray(W_qkv, dtype=np.float32)
    bf = np.asarray(b_qkv, dtype=np.float32)
    Wo = np.asarray(W_out, dtype=np.float32)
    bo = np.asarray(b_out, dtype=np.float32)

    c = _cantor_coords(S)
    perm = np.argsort(c, kind="stable")
    inv = np.empty(S, dtype=np.int64)
    inv[perm] = np.arange(S)

    # smoothed x (commutes with k/v projection)
    xs = 0.5 * x2
    xs[1:] += 0.25 * x2[:-1]
    xs[0] += 0.25 * x2[0]
    xs[:-1] += 0.25 * x2[1:]
    xs[-1] += 0.25 * x2[-1]

    xT = x2.T  # [D, S]
    xsT = xs.T

    rp = inv[r]  # [S, K] key sorted-positions, rows = original query index

    wqs = (Wf[:, 0:D] * 0.125).astype(np.float32)
    wks = Wf[:, D : 2 * D]
    wvs = Wf[:, 2 * D : 3 * D]
    bqs = bf[0:D] * 0.125
    bks = bf[D : 2 * D]
    bvs = bf[2 * D : 3 * D]

    def pack_headpair(Wm):  # [D, 512] -> [128, 2048] per (hp, c) blocks
        out = np.empty((128, 2048), dtype=np.float32)
        for hp in range(4):
            cols = np.r_[2 * hp * 64 : 2 * hp * 64 + 128]
            for cc in range(4):
                blk = Wm[cc * 128 : (cc + 1) * 128, :][:, cols]
                out[:, (hp * 4 + cc) * 128 : (hp * 4 + cc + 1) * 128] = blk
        return out.astype(ml_dtypes.bfloat16)

    wq_r = pack_headpair(wqs)
    wk_r = pack_headpair(wks)
    wv_r = _to_chunked(wvs, 4).astype(ml_dtypes.bfloat16)
    wo_r = _to_chunked(Wo, 4).astype(ml_dtypes.bfloat16)

    bq_r = np.empty((128, 4), dtype=np.float32)
    bk_r = np.empty((128, 4), dtype=np.float32)
    for hp in range(4):
        bq_r[:, hp] = bqs[2 * hp * 64 : 2 * hp * 64 + 128]
        bk_r[:, hp] = bks[2 * hp * 64 : 2 * hp * 64 + 128]
    bv_r = bvs.reshape(1, 512).astype(ml_dtypes.bfloat16)
    bo_r = bo.reshape(1, 512).astype(ml_dtypes.bfloat16)

    # sel for the 1/l broadcast: la rows = [h0,h2,h4,h6, h1,h3,h5,h7]
    sel = np.zeros((8, 512), dtype=np.float32)
    for hp in range(4):
        sel[hp, hp * 128 + 0 : hp * 128 + 64] = 1.0  # even head 2hp -> rows 0..63
        sel[4 + hp, hp * 128 + 64 : hp * 128 + 128] = 1.0  # odd head 2hp+1
    has_bv = bool(np.any(bvs))
    has_bo = bool(np.any(bo))

    in_maps = []
    w0s = []
    for cc in range(NCORES):
        qsel = perm[cc * SS : (cc + 1) * SS]  # original query indices, sorted order
        rq = rp[qsel]  # [SS, K] key sorted-positions
        lo, hi = int(rq.min()), int(rq.max())
        assert hi - lo + 1 <= W, f"core {cc} window {hi - lo + 1} > {W}"
        w0 = min(max(0, lo), S - W)
        w0s.append(w0)
        ksel = perm[w0 : w0 + W]  # original key indices for the window

        xq_c = np.ascontiguousarray(xT[:, qsel])
        xs_c = np.ascontiguousarray(xsT[:, ksel])
        xq_r = _to_chunked(xq_c, 4).astype(ml_dtypes.bfloat16)
        xs_r = _to_chunked(xs_c, 4).astype(ml_dtypes.bfloat16)

        Mloc = np.zeros((W, SS), dtype=np.float32)
        np.add.at(Mloc, (rq - w0, np.arange(SS)[None, :].repeat(64, axis=0).T), 1.0)
        m_r = np.empty((128, NJB * SS), dtype=np.float32)
        for jb in range(NJB):
            m_r[:, jb * SS : (jb + 1) * SS] = Mloc[jb * 128 : (jb + 1) * 128, :]
        in_maps.append(
            {
                "xq": xq_r,
                "xs": xs_r,
                "wq": wq_r,
                "wk": wk_r,
                "wv": wv_r,
                "wo": wo_r,
                "m": m_r.astype(ml_dtypes.bfloat16),
                "sel": sel.astype(ml_dtypes.bfloat16),
                "bq": bq_r,
                "bk": bk_r,
                "bv": bv_r,
                "bo": bo_r,
            }
        )
    return in_maps, perm, has_bv, has_bo


def _run(nc, in_maps, **kw):
    return bass_utils.run_bass_kernel_spmd(nc, in_maps, list(range(NCORES)), **kw)


def kernel(x, routes, W_qkv, b_qkv, W_out, b_out, _timing=None):
    global _nc
    in_maps, perm, has_bv, has_bo = _prep(x, routes, W_qkv, b_qkv, W_out, b_out)
    if _nc is None:
        _nc = _build(has_bv, has_bo)
    r1 = _run(_nc, in_maps)
    ys = np.concatenate([r1.results[c]["y"] for c in range(NCORES)], axis=0)  # [S, D]
    out = np.empty((S, D), dtype=np.float32)
    out[perm] = ys
    if _timing is not None:
        _timing["phases"] = [("fused", _nc, in_maps)]
    return out.reshape(1, S, D).astype(np.float32)
